# revision 7
# baseline (speedup 1.0000x reference)
"""CBOW negative-sampling loss on 8 Trainium2 NeuronCores.

Strategy (from sharding hint): replicate both embedding tables, data-parallel
over the batch dim. Each core handles 2048 of the 16384 batch rows.

Per-core kernel layout:
  - batch row b -> chunk c = b // 128, partition p = b % 128.
  - 16 chunks, processed in 4 groups of 4 chunks; each group does ONE
    indirect-DMA gather from u_emb (128 part x 32 idx x 128 f32) and ONE from
    w_emb (128 part x 24 idx x 128 f32), amortizing SWDGE fixed overhead.
  - per chunk: h = sum of 8 context embeddings (DVE reduce over strided AP),
    scores = per-row dot(h, w_gathered) for [pos, 5 negs] via broadcast-mult +
    reduce; pos score negated in the reduce.
  - Sigmoid then Ln (with accum_out) over all 96 scores/partition gives the
    per-partition sum of log_sigmoid terms [128, 1]; host sums 8x128 partials
    and negates.

loss = -[ sum_b ln(sigmoid(score_b)) + sum_{b,k} ln(sigmoid(-neg_score_bk)) ]
"""

import sys

import numpy as np

sys.path.insert(0, "/opt/trn_rl_repo")

from concourse import bacc, bass, mybir, tile  # noqa: E402
from concourse.bass_utils import run_bass_kernel_spmd  # noqa: E402

V, D = 100000, 128
B, C, K = 16384, 8, 5
N_CORES = 8
P = 128
B_LOC = B // N_CORES            # 2048 batch rows per core
N_CHUNK = B_LOC // P            # 16 chunks of 128 rows
GROUP = 4                       # chunks per indirect-DMA gather
N_GROUP = N_CHUNK // GROUP      # 4
UW = GROUP * C                  # 32 u-indices per partition per group
J = 1 + K                       # 6 w-rows per batch row (pos + negs)
WW = GROUP * J                  # 24 w-indices per partition per group
NS = N_CHUNK * J                # 96 score columns per partition

_NC_CACHE = {}


def _build_bass():
    nc = bacc.Bacc("TRN2", target_bir_lowering=False, debug=False)

    u_emb = nc.dram_tensor("u_emb", [V, D], mybir.dt.float32, kind="ExternalInput")
    w_emb = nc.dram_tensor("w_emb", [V, D], mybir.dt.float32, kind="ExternalInput")
    uidx = nc.dram_tensor("uidx", [N_GROUP, P, UW], mybir.dt.int32, kind="ExternalInput")
    widx = nc.dram_tensor("widx", [N_GROUP, P, WW], mybir.dt.int32, kind="ExternalInput")
    loss = nc.dram_tensor("loss_part", [P, 1], mybir.dt.float32, kind="ExternalOutput")

    fp32 = mybir.dt.float32
    X = mybir.AxisListType.X
    ADD = mybir.AluOpType.add

    with tile.TileContext(nc) as tc:
        with (
            tc.tile_pool(name="idx", bufs=2) as idx_pool,
            tc.tile_pool(name="ug", bufs=2) as ug_pool,
            tc.tile_pool(name="wg", bufs=2) as wg_pool,
            tc.tile_pool(name="m", bufs=2) as m_pool,
            tc.tile_pool(name="fin", bufs=1) as fin_pool,
        ):
            scores = fin_pool.tile([P, NS], fp32, tag="scores")
            for g in range(N_GROUP):
                uix = idx_pool.tile([P, UW], mybir.dt.int32, tag="uix")
                wix = idx_pool.tile([P, WW], mybir.dt.int32, tag="wix")
                nc.sync.dma_start(out=uix[:], in_=uidx[g])
                nc.sync.dma_start(out=wix[:], in_=widx[g])

                ug = ug_pool.tile([P, UW * D], fp32, tag="ug")
                wg = wg_pool.tile([P, WW * D], fp32, tag="wg")
                nc.gpsimd.indirect_dma_start(
                    out=ug[:],
                    out_offset=None,
                    in_=u_emb[:],
                    in_offset=bass.IndirectOffsetOnAxis(ap=uix[:], axis=0),
                )
                nc.gpsimd.indirect_dma_start(
                    out=wg[:],
                    out_offset=None,
                    in_=w_emb[:],
                    in_offset=bass.IndirectOffsetOnAxis(ap=wix[:], axis=0),
                )

                for c in range(GROUP):
                    chunk = g * GROUP + c
                    # h = sum over the 8 context embeddings. In-place binary
                    # tree of contiguous adds inside this chunk's gather block
                    # (strided reduce over [d, k] was ~2x slower on DVE).
                    b0 = c * C * D
                    for width in (C * D // 2, C * D // 4, C * D // 8):
                        nc.vector.tensor_add(
                            out=ug[:, b0 : b0 + width],
                            in0=ug[:, b0 : b0 + width],
                            in1=ug[:, b0 + width : b0 + 2 * width],
                        )
                    h = ug[:, b0 : b0 + D]

                    # m[p, j, d] = w_gathered[p, j, d] * h[p, d]
                    m = m_pool.tile([P, J * D], fp32, tag="m")
                    w_view = wg[:, c * J * D : (c + 1) * J * D].rearrange(
                        "p (j d) -> p j d", j=J
                    )
                    m_view = m[:].rearrange("p (j d) -> p j d", j=J)
                    nc.vector.tensor_mul(
                        out=m_view,
                        in0=w_view,
                        in1=h[:, None, :].broadcast_to([P, J, D]),
                    )

                    # scores[:, 6*chunk]     = +dot(h, w_pos)
                    # scores[:, 6*chunk+1:6] = -dot(h, w_negk)
                    # so each entry x contributes log_sigmoid(x) to the
                    # (negated) loss; host computes loss = -sum(ln(sigmoid(x)))
                    s0 = J * chunk
                    nc.vector.tensor_reduce(
                        out=scores[:, s0 : s0 + 1],
                        in_=m_view[:, 0:1, :],
                        axis=X,
                        op=ADD,
                    )
                    nc.vector.tensor_reduce(
                        out=scores[:, s0 + 1 : s0 + J],
                        in_=m_view[:, 1:J, :],
                        axis=X,
                        op=ADD,
                        negate=True,
                    )

            sg = fin_pool.tile([P, NS], fp32, tag="sg")
            sp = fin_pool.tile([P, NS], fp32, tag="sp")
            lp = fin_pool.tile([P, 1], fp32, tag="lp")
            nc.scalar.activation(
                out=sg[:],
                in_=scores[:],
                func=mybir.ActivationFunctionType.Sigmoid,
            )
            nc.scalar.activation(
                out=sp[:],
                in_=sg[:],
                func=mybir.ActivationFunctionType.Ln,
                accum_out=lp[:],
            )
            nc.sync.dma_start(out=loss[:], in_=lp[:])

    nc.compile()
    return nc


def _get_nc():
    if "nc" not in _NC_CACHE:
        _NC_CACHE["nc"] = _build_bass()
    return _NC_CACHE["nc"]


def _make_in_maps(pos_u, pos_w, neg_w, u_emb, w_emb):
    pos_u = np.ascontiguousarray(np.asarray(pos_u).astype(np.int32))
    pos_w = np.ascontiguousarray(np.asarray(pos_w).astype(np.int32))
    neg_w = np.ascontiguousarray(np.asarray(neg_w).astype(np.int32))
    u_emb = np.ascontiguousarray(np.asarray(u_emb, dtype=np.float32))
    w_emb = np.ascontiguousarray(np.asarray(w_emb, dtype=np.float32))

    in_maps = []
    for i in range(N_CORES):
        sl = slice(i * B_LOC, (i + 1) * B_LOC)
        pu = pos_u[sl]                                        # [2048, 8]
        wi = np.concatenate([pos_w[sl, None], neg_w[sl]], 1)  # [2048, 6]
        # batch row b -> (group g, sub-chunk c, partition p): b = (4g+c)*128+p
        uidx = (
            pu.reshape(N_GROUP, GROUP, P, C).transpose(0, 2, 1, 3).reshape(N_GROUP, P, UW)
        )
        widx = (
            wi.reshape(N_GROUP, GROUP, P, J).transpose(0, 2, 1, 3).reshape(N_GROUP, P, WW)
        )
        in_maps.append(
            {
                "u_emb": u_emb,
                "w_emb": w_emb,
                "uidx": np.ascontiguousarray(uidx),
                "widx": np.ascontiguousarray(widx),
            }
        )
    return in_maps


def _install_axon_profile_shim():
    """Provide antenv.axon_hooks (missing in this image) so trace=True can
    capture NTFF profiles via the axon PJRT .so, and keep trace artifacts
    local instead of uploading to a bucket."""
    import contextlib
    import ctypes
    import types

    import concourse.bass_utils as bu

    bu.upload_artifacts = lambda tmpdir: tmpdir

    try:
        from antenv.axon_hooks import get_axon_ntff_profile_hook  # noqa: F401

        return
    except ImportError:
        pass

    mod = types.ModuleType("antenv.axon_hooks")
    holder = {}
    mod.set_axon_ntff_profile_hook = lambda h: holder.__setitem__("h", h)
    mod.get_axon_ntff_profile_hook = lambda: holder.get("h")
    sys.modules["antenv.axon_hooks"] = mod
    import antenv

    antenv.axon_hooks = mod

    so_path = "/opt/axon/libaxon_pjrt.so"
    lib = ctypes.CDLL(so_path)
    if not hasattr(lib, "axon_start_nrt_profile"):
        return
    lib.axon_start_nrt_profile.argtypes = [
        ctypes.POINTER(ctypes.c_int64),
        ctypes.c_size_t,
    ]
    lib.axon_start_nrt_profile.restype = ctypes.c_int64
    lib.axon_stop_nrt_profile.argtypes = [ctypes.c_char_p]
    lib.axon_stop_nrt_profile.restype = ctypes.c_int64

    @contextlib.contextmanager
    def _hook(output_dir, device_ids):
        import jax

        jax.devices()
        if device_ids:
            ids = (ctypes.c_int64 * len(device_ids))(*device_ids)
            rc = lib.axon_start_nrt_profile(ids, len(device_ids))
        else:
            rc = lib.axon_start_nrt_profile(None, 0)
        if rc != 0:
            raise RuntimeError(f"axon_start_nrt_profile rc={rc}")
        try:
            yield
        finally:
            n = lib.axon_stop_nrt_profile(str(output_dir).encode())
            print(f"profile: {n} file(s) written to {output_dir}")

    mod.set_axon_ntff_profile_hook(_hook)


def _run(in_maps, trace=False):
    if trace:
        _install_axon_profile_shim()
    nc = _get_nc()
    return run_bass_kernel_spmd(nc, in_maps, list(range(N_CORES)), trace=trace)


def kernel(pos_u, pos_w, neg_w, u_emb, w_emb):
    in_maps = _make_in_maps(pos_u, pos_w, neg_w, u_emb, w_emb)
    bkr = _run(in_maps, trace=False)
    total = 0.0
    for r in bkr.results:
        total += float(r["loss_part"].astype(np.float64).sum())
    return np.float32(-total)


def kernel_traced(pos_u, pos_w, neg_w, u_emb, w_emb):
    """Like kernel() but returns (loss, BassKernelResults) with HW profile."""
    in_maps = _make_in_maps(pos_u, pos_w, neg_w, u_emb, w_emb)
    bkr = _run(in_maps, trace=True)
    total = 0.0
    for r in bkr.results:
        total += float(r["loss_part"].astype(np.float64).sum())
    return np.float32(-total), bkr


# revision 8
# speedup vs baseline: 1.0376x; 1.0376x over previous
"""CBOW negative-sampling loss on 8 Trainium2 NeuronCores.

Strategy (from sharding hint): replicate both embedding tables, data-parallel
over the batch dim. Each core handles 2048 of the 16384 batch rows.

Per-core kernel layout:
  - batch row b -> chunk c = b // 128, partition p = b % 128.
  - 16 chunks, processed in 4 groups of 4 chunks; each group does ONE
    indirect-DMA gather from u_emb (128 part x 32 idx x 128 f32) and ONE from
    w_emb (128 part x 24 idx x 128 f32), amortizing SWDGE fixed overhead.
  - per chunk: h = sum of 8 context embeddings (DVE reduce over strided AP),
    scores = per-row dot(h, w_gathered) for [pos, 5 negs] via broadcast-mult +
    reduce; pos score negated in the reduce.
  - Sigmoid then Ln (with accum_out) over all 96 scores/partition gives the
    per-partition sum of log_sigmoid terms [128, 1]; host sums 8x128 partials
    and negates.

loss = -[ sum_b ln(sigmoid(score_b)) + sum_{b,k} ln(sigmoid(-neg_score_bk)) ]
"""

import sys

import numpy as np

sys.path.insert(0, "/opt/trn_rl_repo")

from concourse import bacc, bass, mybir, tile  # noqa: E402
from concourse.bass_utils import run_bass_kernel_spmd  # noqa: E402

V, D = 100000, 128
B, C, K = 16384, 8, 5
N_CORES = 8
P = 128
B_LOC = B // N_CORES            # 2048 batch rows per core
N_CHUNK = B_LOC // P            # 16 chunks of 128 rows
GROUP = 4                       # chunks per indirect-DMA gather
N_GROUP = N_CHUNK // GROUP      # 4
UW = GROUP * C                  # 32 u-indices per partition per group
J = 1 + K                       # 6 w-rows per batch row (pos + negs)
WW = GROUP * J                  # 24 w-indices per partition per group
NS = N_CHUNK * J                # 96 score columns per partition

_NC_CACHE = {}


def _build_bass():
    nc = bacc.Bacc("TRN2", target_bir_lowering=False, debug=False)

    u_emb = nc.dram_tensor("u_emb", [V, D], mybir.dt.float32, kind="ExternalInput")
    w_emb = nc.dram_tensor("w_emb", [V, D], mybir.dt.float32, kind="ExternalInput")
    uidx = nc.dram_tensor("uidx", [N_GROUP, P, UW], mybir.dt.int32, kind="ExternalInput")
    widx = nc.dram_tensor("widx", [N_GROUP, P, WW], mybir.dt.int32, kind="ExternalInput")
    loss = nc.dram_tensor("loss_part", [P, 1], mybir.dt.float32, kind="ExternalOutput")

    fp32 = mybir.dt.float32
    X = mybir.AxisListType.X
    ADD = mybir.AluOpType.add

    with tile.TileContext(nc) as tc:
        with (
            tc.tile_pool(name="idx", bufs=2) as idx_pool,
            tc.tile_pool(name="ug", bufs=2) as ug_pool,
            tc.tile_pool(name="wg", bufs=2) as wg_pool,
            tc.tile_pool(name="m", bufs=2) as m_pool,
            tc.tile_pool(name="fin", bufs=1) as fin_pool,
        ):
            scores = fin_pool.tile([P, NS], fp32, tag="scores")
            # sign pattern for the score columns: [+1, -1, -1, -1, -1, -1]
            pat = fin_pool.tile([P, J], fp32, tag="pat")
            nc.gpsimd.memset(pat[:], -1.0)
            nc.gpsimd.memset(pat[:, 0:1], 1.0)

            ug_t, wg_t = {}, {}

            def issue_gather(g):
                uix = idx_pool.tile([P, UW], mybir.dt.int32, tag="uix")
                wix = idx_pool.tile([P, WW], mybir.dt.int32, tag="wix")
                nc.sync.dma_start(out=uix[:], in_=uidx[g])
                nc.sync.dma_start(out=wix[:], in_=widx[g])
                ug = ug_pool.tile([P, UW * D], fp32, tag="ug")
                wg = wg_pool.tile([P, WW * D], fp32, tag="wg")
                nc.gpsimd.indirect_dma_start(
                    out=ug[:],
                    out_offset=None,
                    in_=u_emb[:],
                    in_offset=bass.IndirectOffsetOnAxis(ap=uix[:], axis=0),
                )
                nc.gpsimd.indirect_dma_start(
                    out=wg[:],
                    out_offset=None,
                    in_=w_emb[:],
                    in_offset=bass.IndirectOffsetOnAxis(ap=wix[:], axis=0),
                )
                ug_t[g], wg_t[g] = ug, wg

            issue_gather(0)
            for g in range(N_GROUP):
                if g + 1 < N_GROUP:
                    issue_gather(g + 1)
                ug, wg = ug_t.pop(g), wg_t.pop(g)

                # h = sum of the 8 context embeddings per chunk; binary tree
                # of contiguous adds, all 4 chunks of the group per
                # instruction. First (largest) level runs on GpSimd to take
                # load off DVE.
                ug3 = ug[:].rearrange("p (c e) -> p c e", c=GROUP)
                nc.gpsimd.tensor_add(
                    out=ug3[:, :, 0 : 4 * D],
                    in0=ug3[:, :, 0 : 4 * D],
                    in1=ug3[:, :, 4 * D : 8 * D],
                )
                nc.vector.tensor_add(
                    out=ug3[:, :, 0 : 2 * D],
                    in0=ug3[:, :, 0 : 2 * D],
                    in1=ug3[:, :, 2 * D : 4 * D],
                )
                nc.vector.tensor_add(
                    out=ug3[:, :, 0:D],
                    in0=ug3[:, :, 0:D],
                    in1=ug3[:, :, D : 2 * D],
                )
                h4 = ug3[:, :, 0:D]  # [P, GROUP, D]

                # m[p, c, j, d] = w[p, c, j, d] * h[p, c, d]
                m = m_pool.tile([P, GROUP * J * D], fp32, tag="m")
                m4 = m[:].rearrange("p (c j d) -> p c j d", c=GROUP, j=J)
                w4 = wg[:].rearrange("p (c j d) -> p c j d", c=GROUP, j=J)
                nc.vector.tensor_mul(
                    out=m4,
                    in0=w4,
                    in1=h4[:, :, None, :].broadcast_to([P, GROUP, J, D]),
                )
                # raw dots for the whole group -> scores[:, g*24 : (g+1)*24]
                nc.vector.tensor_reduce(
                    out=scores[:, g * GROUP * J : (g + 1) * GROUP * J],
                    in_=m4,
                    axis=X,
                    op=ADD,
                )

            # flip sign of the neg-sample columns: x -> log_sigmoid operand
            sc3 = scores[:].rearrange("p (t j) -> p t j", j=J)
            nc.vector.tensor_mul(
                out=sc3,
                in0=sc3,
                in1=pat[:, None, :].broadcast_to([P, N_CHUNK, J]),
            )

            sg = fin_pool.tile([P, NS], fp32, tag="sg")
            sp = fin_pool.tile([P, NS], fp32, tag="sp")
            lp = fin_pool.tile([P, 1], fp32, tag="lp")
            nc.scalar.activation(
                out=sg[:],
                in_=scores[:],
                func=mybir.ActivationFunctionType.Sigmoid,
            )
            nc.scalar.activation(
                out=sp[:],
                in_=sg[:],
                func=mybir.ActivationFunctionType.Ln,
                accum_out=lp[:],
            )
            nc.sync.dma_start(out=loss[:], in_=lp[:])

    nc.compile()
    return nc


def _get_nc():
    if "nc" not in _NC_CACHE:
        _NC_CACHE["nc"] = _build_bass()
    return _NC_CACHE["nc"]


def _make_in_maps(pos_u, pos_w, neg_w, u_emb, w_emb):
    pos_u = np.ascontiguousarray(np.asarray(pos_u).astype(np.int32))
    pos_w = np.ascontiguousarray(np.asarray(pos_w).astype(np.int32))
    neg_w = np.ascontiguousarray(np.asarray(neg_w).astype(np.int32))
    u_emb = np.ascontiguousarray(np.asarray(u_emb, dtype=np.float32))
    w_emb = np.ascontiguousarray(np.asarray(w_emb, dtype=np.float32))

    in_maps = []
    for i in range(N_CORES):
        sl = slice(i * B_LOC, (i + 1) * B_LOC)
        pu = pos_u[sl]                                        # [2048, 8]
        wi = np.concatenate([pos_w[sl, None], neg_w[sl]], 1)  # [2048, 6]
        # batch row b -> (group g, sub-chunk c, partition p): b = (4g+c)*128+p
        uidx = (
            pu.reshape(N_GROUP, GROUP, P, C).transpose(0, 2, 1, 3).reshape(N_GROUP, P, UW)
        )
        widx = (
            wi.reshape(N_GROUP, GROUP, P, J).transpose(0, 2, 1, 3).reshape(N_GROUP, P, WW)
        )
        in_maps.append(
            {
                "u_emb": u_emb,
                "w_emb": w_emb,
                "uidx": np.ascontiguousarray(uidx),
                "widx": np.ascontiguousarray(widx),
            }
        )
    return in_maps


def _install_axon_profile_shim():
    """Provide antenv.axon_hooks (missing in this image) so trace=True can
    capture NTFF profiles via the axon PJRT .so, and keep trace artifacts
    local instead of uploading to a bucket."""
    import contextlib
    import ctypes
    import types

    import concourse.bass_utils as bu

    bu.upload_artifacts = lambda tmpdir: tmpdir

    try:
        from antenv.axon_hooks import get_axon_ntff_profile_hook  # noqa: F401

        return
    except ImportError:
        pass

    mod = types.ModuleType("antenv.axon_hooks")
    holder = {}
    mod.set_axon_ntff_profile_hook = lambda h: holder.__setitem__("h", h)
    mod.get_axon_ntff_profile_hook = lambda: holder.get("h")
    sys.modules["antenv.axon_hooks"] = mod
    import antenv

    antenv.axon_hooks = mod

    so_path = "/opt/axon/libaxon_pjrt.so"
    lib = ctypes.CDLL(so_path)
    if not hasattr(lib, "axon_start_nrt_profile"):
        return
    lib.axon_start_nrt_profile.argtypes = [
        ctypes.POINTER(ctypes.c_int64),
        ctypes.c_size_t,
    ]
    lib.axon_start_nrt_profile.restype = ctypes.c_int64
    lib.axon_stop_nrt_profile.argtypes = [ctypes.c_char_p]
    lib.axon_stop_nrt_profile.restype = ctypes.c_int64

    @contextlib.contextmanager
    def _hook(output_dir, device_ids):
        import jax

        jax.devices()
        if device_ids:
            ids = (ctypes.c_int64 * len(device_ids))(*device_ids)
            rc = lib.axon_start_nrt_profile(ids, len(device_ids))
        else:
            rc = lib.axon_start_nrt_profile(None, 0)
        if rc != 0:
            raise RuntimeError(f"axon_start_nrt_profile rc={rc}")
        try:
            yield
        finally:
            n = lib.axon_stop_nrt_profile(str(output_dir).encode())
            print(f"profile: {n} file(s) written to {output_dir}")

    mod.set_axon_ntff_profile_hook(_hook)


def _run(in_maps, trace=False):
    if trace:
        _install_axon_profile_shim()
    nc = _get_nc()
    return run_bass_kernel_spmd(nc, in_maps, list(range(N_CORES)), trace=trace)


def kernel(pos_u, pos_w, neg_w, u_emb, w_emb):
    in_maps = _make_in_maps(pos_u, pos_w, neg_w, u_emb, w_emb)
    bkr = _run(in_maps, trace=False)
    total = 0.0
    for r in bkr.results:
        total += float(r["loss_part"].astype(np.float64).sum())
    return np.float32(-total)


def kernel_traced(pos_u, pos_w, neg_w, u_emb, w_emb):
    """Like kernel() but returns (loss, BassKernelResults) with HW profile."""
    in_maps = _make_in_maps(pos_u, pos_w, neg_w, u_emb, w_emb)
    bkr = _run(in_maps, trace=True)
    total = 0.0
    for r in bkr.results:
        total += float(r["loss_part"].astype(np.float64).sum())
    return np.float32(-total), bkr


# revision 9
# speedup vs baseline: 1.0917x; 1.0522x over previous
"""CBOW negative-sampling loss on 8 Trainium2 NeuronCores.

Strategy (from sharding hint): replicate both embedding tables, data-parallel
over the batch dim. Each core handles 2048 of the 16384 batch rows.

Host side: u_emb and w_emb are concatenated into one [2V, D] table so each
group needs a single indirect-DMA gather; w-indices are offset by +V.

Per-core kernel layout:
  - batch row b -> chunk c = b // 128, partition p = b % 128.
  - 16 chunks in 4 groups of 4; per group ONE indirect gather pulls, for each
    partition, 4 chunks x (8 u-rows + 1 pos-w + 5 neg-w rows) x 128 f32.
  - h = sum of the 8 context embeddings: binary tree of contiguous adds over
    all 4 chunks at once (level 1 on GpSimd, levels 2-3 on DVE).
  - dots: one broadcast-mult [P,4,6,128] + one X-reduce -> 24 score cols/group.
  - sign pattern [+1,-1,-1,-1,-1,-1] applied once to all 96 cols.
  - Exp(-x) then Ln(in+1) with accum_out (both in one ACT table) gives
    per-partition sum of softplus(-x) = the positive partial loss [128, 1].

loss = sum_b softplus(-score_b) + sum_{b,k} softplus(+neg_score_bk)
"""

import sys

import numpy as np

sys.path.insert(0, "/opt/trn_rl_repo")

from concourse import bacc, bass, mybir, tile  # noqa: E402
from concourse.bass_utils import run_bass_kernel_spmd  # noqa: E402

V, D = 100000, 128
B, C, K = 16384, 8, 5
N_CORES = 8
P = 128
B_LOC = B // N_CORES            # 2048 batch rows per core
N_CHUNK = B_LOC // P            # 16 chunks of 128 rows
GROUP = 4                       # chunks per indirect-DMA gather
N_GROUP = N_CHUNK // GROUP      # 4
J = 1 + K                       # 6 w-rows per batch row (pos + negs)
R = C + J                       # 14 gathered rows per batch row
GW = GROUP * R                  # 56 indices per partition per group
NS = N_CHUNK * J                # 96 score columns per partition

_NC_CACHE = {}


def _build_bass():
    nc = bacc.Bacc(
        "TRN2",
        target_bir_lowering=False,
        debug=False,
        dynamic_dma_scratch_size=65536,
    )

    emb = nc.dram_tensor("emb_cat", [2 * V, D], mybir.dt.float32, kind="ExternalInput")
    gidx = nc.dram_tensor("gidx", [N_GROUP, P, GW], mybir.dt.int32, kind="ExternalInput")
    loss = nc.dram_tensor("loss_part", [P, 1], mybir.dt.float32, kind="ExternalOutput")

    fp32 = mybir.dt.float32
    X = mybir.AxisListType.X
    ADD = mybir.AluOpType.add

    with tile.TileContext(nc) as tc:
        with (
            tc.tile_pool(name="idx", bufs=3) as idx_pool,
            tc.tile_pool(name="gb", bufs=3) as gb_pool,
            tc.tile_pool(name="m", bufs=2) as m_pool,
            tc.tile_pool(name="fin", bufs=1) as fin_pool,
        ):
            scores = fin_pool.tile([P, NS], fp32, tag="scores")
            # sign pattern per 6 score cols: [+1, -1, -1, -1, -1, -1]
            pat = fin_pool.tile([P, J], fp32, tag="pat")
            nc.gpsimd.memset(pat[:], -1.0)
            nc.gpsimd.memset(pat[:, 0:1], 1.0)

            gb_t = {}

            def issue_gather(g):
                ix = idx_pool.tile([P, GW], mybir.dt.int32, tag="ix")
                nc.sync.dma_start(out=ix[:], in_=gidx[g])
                gb = gb_pool.tile([P, GW * D], fp32, tag="gb")
                nc.gpsimd.indirect_dma_start(
                    out=gb[:],
                    out_offset=None,
                    in_=emb[:],
                    in_offset=bass.IndirectOffsetOnAxis(ap=ix[:], axis=0),
                )
                gb_t[g] = gb

            issue_gather(0)
            for g in range(N_GROUP):
                if g + 1 < N_GROUP:
                    issue_gather(g + 1)
                gb = gb_t.pop(g)
                g3 = gb[:].rearrange("p (c e) -> p c e", c=GROUP)  # e = R*D

                # h = sum of the 8 context embeddings (cols 0 : 8D of each
                # chunk block); contiguous binary tree, level 1 on GpSimd.
                nc.gpsimd.tensor_add(
                    out=g3[:, :, 0 : 4 * D],
                    in0=g3[:, :, 0 : 4 * D],
                    in1=g3[:, :, 4 * D : 8 * D],
                )
                nc.vector.tensor_add(
                    out=g3[:, :, 0 : 2 * D],
                    in0=g3[:, :, 0 : 2 * D],
                    in1=g3[:, :, 2 * D : 4 * D],
                )
                nc.vector.tensor_add(
                    out=g3[:, :, 0:D],
                    in0=g3[:, :, 0:D],
                    in1=g3[:, :, D : 2 * D],
                )
                h4 = g3[:, :, 0:D]  # [P, GROUP, D]

                # m[p, c, j, d] = w[p, c, j, d] * h[p, c, d]
                w4 = g3[:, :, C * D : R * D].rearrange(
                    "p c (j d) -> p c j d", j=J
                )
                m = m_pool.tile([P, GROUP * J * D], fp32, tag="m")
                m4 = m[:].rearrange("p (c j d) -> p c j d", c=GROUP, j=J)
                nc.vector.tensor_mul(
                    out=m4,
                    in0=w4,
                    in1=h4[:, :, None, :].broadcast_to([P, GROUP, J, D]),
                )
                # raw dots for the whole group -> scores[:, g*24 : (g+1)*24]
                nc.vector.tensor_reduce(
                    out=scores[:, g * GROUP * J : (g + 1) * GROUP * J],
                    in_=m4,
                    axis=X,
                    op=ADD,
                )

            # x = [+pos_score, -neg_scores...]; loss term = softplus(-x)
            sc3 = scores[:].rearrange("p (t j) -> p t j", j=J)
            nc.vector.tensor_mul(
                out=sc3,
                in0=sc3,
                in1=pat[:, None, :].broadcast_to([P, N_CHUNK, J]),
            )

            # softplus(-x) = ln(1 + exp(-x)); Exp and Ln share one ACT table
            ex = fin_pool.tile([P, NS], fp32, tag="ex")
            sp = fin_pool.tile([P, NS], fp32, tag="sp")
            lp = fin_pool.tile([P, 1], fp32, tag="lp")
            nc.scalar.activation(
                out=ex[:],
                in_=scores[:],
                func=mybir.ActivationFunctionType.Exp,
                scale=-1.0,
            )
            nc.scalar.activation(
                out=sp[:],
                in_=ex[:],
                func=mybir.ActivationFunctionType.Ln,
                bias=1.0,
                accum_out=lp[:],
            )
            nc.sync.dma_start(out=loss[:], in_=lp[:])

    nc.compile()
    return nc


def _get_nc():
    if "nc" not in _NC_CACHE:
        _NC_CACHE["nc"] = _build_bass()
    return _NC_CACHE["nc"]


def _make_in_maps(pos_u, pos_w, neg_w, u_emb, w_emb):
    pos_u = np.asarray(pos_u).astype(np.int32)
    pos_w = np.asarray(pos_w).astype(np.int32)
    neg_w = np.asarray(neg_w).astype(np.int32)
    u_emb = np.asarray(u_emb, dtype=np.float32)
    w_emb = np.asarray(w_emb, dtype=np.float32)

    emb_cat = np.ascontiguousarray(np.concatenate([u_emb, w_emb], axis=0))

    in_maps = []
    for i in range(N_CORES):
        sl = slice(i * B_LOC, (i + 1) * B_LOC)
        # per batch row: [8 ctx u-idx | pos_w + V | neg_w + V]  -> R = 14
        rows = np.concatenate(
            [pos_u[sl], pos_w[sl, None] + V, neg_w[sl] + V], axis=1
        )  # [B_LOC, 14]
        # batch row b -> (group g, sub-chunk c, partition p): b = (4g+c)*128+p
        gidx = (
            rows.reshape(N_GROUP, GROUP, P, R)
            .transpose(0, 2, 1, 3)
            .reshape(N_GROUP, P, GW)
        )
        in_maps.append(
            {
                "emb_cat": emb_cat,
                "gidx": np.ascontiguousarray(gidx),
            }
        )
    return in_maps


def _install_axon_profile_shim():
    """Provide antenv.axon_hooks (missing in this image) so trace=True can
    capture NTFF profiles via the axon PJRT .so, and keep trace artifacts
    local instead of uploading to a bucket."""
    import contextlib
    import ctypes
    import types

    import concourse.bass_utils as bu

    bu.upload_artifacts = lambda tmpdir: tmpdir

    try:
        from antenv.axon_hooks import get_axon_ntff_profile_hook  # noqa: F401

        return
    except ImportError:
        pass

    mod = types.ModuleType("antenv.axon_hooks")
    holder = {}
    mod.set_axon_ntff_profile_hook = lambda h: holder.__setitem__("h", h)
    mod.get_axon_ntff_profile_hook = lambda: holder.get("h")
    sys.modules["antenv.axon_hooks"] = mod
    import antenv

    antenv.axon_hooks = mod

    so_path = "/opt/axon/libaxon_pjrt.so"
    lib = ctypes.CDLL(so_path)
    if not hasattr(lib, "axon_start_nrt_profile"):
        return
    lib.axon_start_nrt_profile.argtypes = [
        ctypes.POINTER(ctypes.c_int64),
        ctypes.c_size_t,
    ]
    lib.axon_start_nrt_profile.restype = ctypes.c_int64
    lib.axon_stop_nrt_profile.argtypes = [ctypes.c_char_p]
    lib.axon_stop_nrt_profile.restype = ctypes.c_int64

    @contextlib.contextmanager
    def _hook(output_dir, device_ids):
        import jax

        jax.devices()
        if device_ids:
            ids = (ctypes.c_int64 * len(device_ids))(*device_ids)
            rc = lib.axon_start_nrt_profile(ids, len(device_ids))
        else:
            rc = lib.axon_start_nrt_profile(None, 0)
        if rc != 0:
            raise RuntimeError(f"axon_start_nrt_profile rc={rc}")
        try:
            yield
        finally:
            n = lib.axon_stop_nrt_profile(str(output_dir).encode())
            print(f"profile: {n} file(s) written to {output_dir}")

    mod.set_axon_ntff_profile_hook(_hook)


def _run(in_maps, trace=False):
    if trace:
        _install_axon_profile_shim()
    nc = _get_nc()
    return run_bass_kernel_spmd(nc, in_maps, list(range(N_CORES)), trace=trace)


def kernel(pos_u, pos_w, neg_w, u_emb, w_emb):
    in_maps = _make_in_maps(pos_u, pos_w, neg_w, u_emb, w_emb)
    bkr = _run(in_maps, trace=False)
    total = 0.0
    for r in bkr.results:
        total += float(r["loss_part"].astype(np.float64).sum())
    return np.float32(total)


def kernel_traced(pos_u, pos_w, neg_w, u_emb, w_emb):
    """Like kernel() but returns (loss, BassKernelResults) with HW profile."""
    in_maps = _make_in_maps(pos_u, pos_w, neg_w, u_emb, w_emb)
    bkr = _run(in_maps, trace=True)
    total = 0.0
    for r in bkr.results:
        total += float(r["loss_part"].astype(np.float64).sum())
    return np.float32(total), bkr
